# revision 2
# baseline (speedup 1.0000x reference)
"""Trainium2 Bass kernel for nn_AttentionBlock (GroupNorm + 1x1-conv QKV +
softmax attention + 1x1-conv proj + residual), B=4 C=512 H=W=64 HEADS=8.

Sharding: 8 cores = (batch b in 0..4) x (query-half ih in 0..2).  Each core
computes groupnorm + K/V for its whole batch (duplicated across the 2 cores
sharing a batch -- cheap) and attention + proj for its 2048 queries.  Cores
are fully independent SPMD (no collectives); the host splits and concats.
For ih=1 cores the host *rolls* the spatial columns of x by 2048 so that the
query half is always columns [0:2048): groupnorm statistics and the
attention key-sum are permutation-invariant over the spatial axis, so the
math is unchanged.

Per-(b,h) attention uses a "scores transposed" layout S^T[j,i] (keys j on
partitions, queries i on free dim): lhsT=K_h[64d,128j], rhs=Q_h[64d,512i].
Softmax is over the partition dim: exp on ScalarE (PSUM->SBUF, 1/8 scale
folded in; no max subtraction -- logits are bounded ~|1.5| here), column
sums come free from a ones-row appended to the V^T stationary operand
(output row 64 = colsum), and 1/colsum is applied to O via a ones-matmul
broadcast + DVE multiply.  Matmul operands are bf16 (f32 PSUM accumulate):
simulated end-to-end quantization error is ~2e-5 relative.
"""

from contextlib import ExitStack

import numpy as np

import concourse.bass as bass
import concourse.tile as tile
import concourse.mybir as mybir
from concourse import bacc
from concourse.bass_utils import run_bass_kernel_spmd

F32 = mybir.dt.float32
F32R = mybir.dt.float32r
BF16 = mybir.dt.bfloat16
EXP = mybir.ActivationFunctionType.Exp
IDENT = mybir.ActivationFunctionType.Identity
SQRT = mybir.ActivationFunctionType.Sqrt

B, C, HH, WW = 4, 512, 64, 64
S = HH * WW              # 4096
HEADS = 8
HD = C // HEADS          # 64
GROUPS = 32
GSIZE = C // GROUPS      # 16 channels per group
EPS = 1e-5
SCALE = 1.0 / 8.0        # 1/sqrt(head_dim)
IHALF = S // 2           # 2048 queries per core
CT = C // 128            # 4 channel tiles
ST = S // 128            # 32 spatial tiles


def build_kernel(reps: int = 1):
    nc = bacc.Bacc("TRN2", target_bir_lowering=False, debug=False)

    x_d = nc.dram_tensor("x", [C, S], F32, kind="ExternalInput").ap()
    qw_d = nc.dram_tensor("qw_t", [C, C], F32, kind="ExternalInput").ap()   # qkv_w[0:512].T
    kw_d = nc.dram_tensor("kw_t", [C, C], F32, kind="ExternalInput").ap()
    vw_d = nc.dram_tensor("vw_t", [C, C], F32, kind="ExternalInput").ap()
    pw_d = nc.dram_tensor("pw_t", [C, C], F32, kind="ExternalInput").ap()   # proj_w.T
    bias_d = nc.dram_tensor("biases", [4, C], F32, kind="ExternalInput").ap()  # qb,kb,vb,pb
    nwb_d = nc.dram_tensor("nwb", [2, C], F32, kind="ExternalInput").ap()      # norm_w, norm_b
    m8_d = nc.dram_tensor("mask8", [128, 8], F32, kind="ExternalInput").ap()
    m8t_d = nc.dram_tensor("mask8t", [8, 128], F32, kind="ExternalInput").ap()
    out_d = nc.dram_tensor("out", [C, IHALF], F32, kind="ExternalOutput").ap()

    with tile.TileContext(nc) as tc:
        with ExitStack() as ctx:
            const = ctx.enter_context(tc.tile_pool(name="const", bufs=1))

            # weight matrices, rearranged [128, kt, c], rounded to bf16
            wts = {}
            with tc.tile_pool(name="wtmp", bufs=2) as wtmp:
                for nm, ap in [("qw", qw_d), ("kw", kw_d), ("vw", vw_d), ("pw", pw_d)]:
                    t_f = wtmp.tile([128, CT, C], F32, tag="wf", name=f"{nm}_f32")
                    nc.sync.dma_start(out=t_f, in_=ap.rearrange("(kt p) c -> p kt c", p=128))
                    t_b = const.tile([128, CT, C], BF16, name=f"{nm}_bf")
                    nc.vector.tensor_copy(t_b, t_f)
                    wts[nm] = t_b

            bias_f = const.tile([1, 4, C], F32)
            nc.sync.dma_start(out=bias_f, in_=bias_d[None, :, :])
            bias_b = const.tile([1, 4, C], BF16)
            nc.vector.tensor_copy(bias_b, bias_f)

            nwb_t = const.tile([128, 2, CT], F32)
            nc.sync.dma_start(out=nwb_t, in_=nwb_d.rearrange("w (kt p) -> p w kt", p=128))

            m8_t = const.tile([128, 8], F32)
            nc.sync.dma_start(out=m8_t, in_=m8_d)
            m8t_t = const.tile([8, 128], F32)
            nc.sync.dma_start(out=m8t_t, in_=m8t_d)

            ones_row = const.tile([1, 512], BF16)
            nc.vector.memset(ones_row, 1.0)
            ones_f = const.tile([1, 64], F32)
            nc.vector.memset(ones_f, 1.0)
            ones_row_r = const.tile([1, 64], F32R)
            nc.vector.tensor_copy(ones_row_r, ones_f)
            eps_t = const.tile([8, 1], F32)
            nc.vector.memset(eps_t, EPS)

            consts = (wts, bias_b, nwb_t, m8_t, m8t_t, ones_row, ones_row_r, eps_t)
            if reps == 1:
                _one_pass(nc, tc, x_d, out_d, consts)
            else:
                with tc.For_i(0, reps, 1):
                    _one_pass(nc, tc, x_d, out_d, consts)
    nc.compile()
    return nc


def _one_pass(nc, tc, x_d, out_d, consts):
    (wts, bias_b, nwb_t, m8_t, m8t_t, ones_row, ones_row_r, eps_t) = consts
    with ExitStack() as ps:
        # =============== P1: groupnorm statistics ===============
        svec = ps.enter_context(tc.tile_pool(name="svec", bufs=1))
        aff_s = svec.tile([128, CT], F32)   # per-channel scale
        aff_t = svec.tile([128, CT], F32)   # per-channel shift

        with tc.tile_pool(name="xin", bufs=2) as xin, \
             tc.tile_pool(name="stat", bufs=2) as stat, \
             tc.tile_pool(name="gn_ps", bufs=1, space="PSUM") as gn_ps:
            gstats = gn_ps.tile([8, CT, 2], F32)
            for t in range(CT):
                x_t = xin.tile([128, S], F32, tag="x")
                nc.sync.dma_start(out=x_t, in_=x_d[t * 128:(t + 1) * 128, :])
                bnst = stat.tile([128, 8, 6], F32, tag="bnst")
                for sg in range(8):
                    nc.vector.bn_stats(out=bnst[:, sg, :], in_=x_t[:, sg * 512:(sg + 1) * 512])
                mv = stat.tile([128, 2], F32, tag="mv")
                nc.vector.bn_aggr(out=mv, in_=bnst)
                st_t = stat.tile([128, 2], F32, tag="st")   # (mean, 2nd moment)
                nc.vector.tensor_copy(st_t[:, 0:1], mv[:, 0:1])
                sq = stat.tile([128, 1], F32, tag="sq")
                nc.vector.tensor_mul(sq, mv[:, 0:1], mv[:, 0:1])
                nc.vector.tensor_add(st_t[:, 1:2], mv[:, 1:2], sq)
                nc.tensor.matmul(gstats[:, t, :], m8_t, st_t, start=True, stop=True)

            gs = stat.tile([8, CT, 2], F32, tag="gs")
            nc.vector.tensor_copy(gs, gstats)
            grp = stat.tile([8, CT, 2], F32, tag="grp")   # (mu_g, rstd_g)
            for t in range(CT):
                nc.scalar.mul(grp[:, t, 0:1], gs[:, t, 0:1], 1.0 / GSIZE)
                e_t = stat.tile([8, 1], F32, tag="e")
                nc.scalar.mul(e_t, gs[:, t, 1:2], 1.0 / GSIZE)
                musq = stat.tile([8, 1], F32, tag="musq")
                nc.vector.tensor_mul(musq, grp[:, t, 0:1], grp[:, t, 0:1])
                var = stat.tile([8, 1], F32, tag="var")
                nc.vector.tensor_sub(var, e_t, musq)
                std = stat.tile([8, 1], F32, tag="std")
                nc.scalar.activation(std, var, SQRT, bias=eps_t)
                nc.vector.reciprocal(grp[:, t, 1:2], std)

            bc_ps = gn_ps.tile([128, CT, 2], F32)
            for t in range(CT):
                nc.tensor.matmul(bc_ps[:, t, :], m8t_t, grp[:, t, :], start=True, stop=True)
            bc = stat.tile([128, CT, 2], F32, tag="bc")
            nc.vector.tensor_copy(bc, bc_ps)
            # aff_s = norm_w * rstd ; aff_t = norm_b - mu * aff_s
            nc.vector.tensor_mul(aff_s, nwb_t[:, 0, :], bc[:, :, 1])
            tmp_mu = stat.tile([128, CT], F32, tag="tmpmu")
            nc.vector.tensor_mul(tmp_mu, bc[:, :, 0], aff_s)
            nc.vector.tensor_sub(aff_t, nwb_t[:, 1, :], tmp_mu)

        # =============== P2: normalize -> h (bf16), qkv projections ===============
        big = ps.enter_context(tc.tile_pool(name="big", bufs=1))
        h_all = big.tile([128, CT, S], BF16)           # 32 KB/part
        k_all = big.tile([128, CT, S], BF16)           # 32 KB/part
        q_all = big.tile([128, CT, IHALF], BF16)       # 16 KB/part
        vt_ext = big.tile([128, ST, HEADS, 66], BF16)  # ~33 KB/part
        o_norm = big.tile([128, CT, IHALF], BF16)      # 16 KB/part

        with tc.tile_pool(name="xin2", bufs=2) as xin2:
            for t in range(CT):
                x_t = xin2.tile([128, S], F32, tag="x2")
                nc.sync.dma_start(out=x_t, in_=x_d[t * 128:(t + 1) * 128, :])
                nc.scalar.activation(h_all[:, t, :], x_t, IDENT,
                                     bias=aff_t[:, t:t + 1], scale=aff_s[:, t:t + 1])

        nc.vector.memset(vt_ext[:, :, :, 64:66], 0.0)
        nc.vector.memset(vt_ext[:, :, :, 64:65], 1.0)

        with tc.tile_pool(name="qkv_ps", bufs=2, space="PSUM") as qkv_ps, \
             tc.tile_pool(name="qkv_sb", bufs=3) as qkv_sb:
            del qkv_sb
            # K: [512c, S] and Q: [512c, IHALF] (query half = cols [0:2048))
            for dst, w_b, bidx, ncols in ((k_all, wts["kw"], 1, S),
                                          (q_all, wts["qw"], 0, IHALF)):
                for mt in range(CT):
                    for icol in range(ncols // 512):
                        acc = qkv_ps.tile([128, 512], F32, tag="qkps")
                        nc.tensor.matmul(acc, bias_b[:, bidx, mt * 128:(mt + 1) * 128],
                                         ones_row, start=True, stop=False)
                        for kt in range(CT):
                            nc.tensor.matmul(acc, w_b[:, kt, mt * 128:(mt + 1) * 128],
                                             h_all[:, kt, icol * 512:(icol + 1) * 512],
                                             start=False, stop=(kt == CT - 1))
                        nc.vector.tensor_copy(dst[:, mt, icol * 512:(icol + 1) * 512], acc)
            # V^T: [S, 512c] scattered into vt_ext (64 cols per head + ones col)
            for st in range(ST):
                acc = qkv_ps.tile([128, 512], F32, tag="qkps")
                nc.tensor.matmul(acc, ones_row[:, 0:128], bias_b[:, 2, :],
                                 start=True, stop=False)
                for kt in range(CT):
                    nc.tensor.matmul(acc, h_all[:, kt, st * 128:(st + 1) * 128],
                                     wts["vw"][:, kt, :], start=False, stop=(kt == CT - 1))
                nc.vector.tensor_copy(vt_ext[:, st, :, 0:64],
                                      acc.rearrange("p (h d) -> p h d", d=HD))

        # =============== P3: attention ===============
        with tc.tile_pool(name="sc_ps", bufs=2, space="PSUM") as sc_psp, \
             tc.tile_pool(name="o_ps", bufs=1, space="PSUM") as o_psp, \
             tc.tile_pool(name="rb_ps", bufs=1, space="PSUM") as rb_psp, \
             tc.tile_pool(name="att_sb", bufs=3) as att_sb, \
             tc.tile_pool(name="exp_sb", bufs=3) as exp_sb:
            for h in range(HEADS):
                kt_h = h // 2
                p0 = 64 * (h % 2)
                for iw in range(2):
                    o_ps = o_psp.tile([65, 1024], F32, tag="ops")
                    for jt in range(ST):
                        sc = sc_psp.tile([128, 1024], F32, tag="sc")
                        for ic in range(2):
                            nc.tensor.matmul(
                                sc[:, ic * 512:(ic + 1) * 512],
                                k_all[p0:p0 + 64, kt_h, jt * 128:(jt + 1) * 128],
                                q_all[p0:p0 + 64, kt_h,
                                      iw * 1024 + ic * 512: iw * 1024 + (ic + 1) * 512],
                                start=True, stop=True)
                        ex = exp_sb.tile([128, 1024], BF16, tag="ex")
                        nc.scalar.activation(ex, sc, EXP, scale=SCALE)
                        for ic in range(2):
                            nc.tensor.matmul(
                                o_ps[:, ic * 512:(ic + 1) * 512],
                                vt_ext[:, jt, h, 0:65],
                                ex[:, ic * 512:(ic + 1) * 512],
                                start=(jt == 0), stop=(jt == ST - 1))
                    # normalize by colsum (row 64)
                    o_sb = att_sb.tile([65, 1024], F32, tag="osb")
                    nc.vector.tensor_copy(o_sb, o_ps)
                    r_f = att_sb.tile([1, 1024], F32, tag="rf")
                    nc.vector.reciprocal(r_f, o_sb[64:65, :])
                    r_r = att_sb.tile([1, 1024], F32R, tag="rr")
                    nc.vector.tensor_copy(r_r, r_f)
                    rb = rb_psp.tile([64, 1024], F32, tag="rb")
                    for ic in range(2):
                        nc.tensor.matmul(rb[:, ic * 512:(ic + 1) * 512], ones_row_r,
                                         r_r[:, ic * 512:(ic + 1) * 512],
                                         start=True, stop=True)
                    nc.vector.tensor_mul(
                        o_norm[p0:p0 + 64, kt_h, iw * 1024:(iw + 1) * 1024],
                        o_sb[0:64, :], rb)

        # =============== P4: proj + residual ===============
        with tc.tile_pool(name="pj_ps", bufs=2, space="PSUM") as pj_ps, \
             tc.tile_pool(name="pj_sb", bufs=3) as pj_sb, \
             tc.tile_pool(name="xres", bufs=3) as xres:
            for mt in range(CT):
                for icol in range(IHALF // 512):
                    acc = pj_ps.tile([128, 512], F32, tag="pj")
                    nc.tensor.matmul(acc, bias_b[:, 3, mt * 128:(mt + 1) * 128],
                                     ones_row, start=True, stop=False)
                    for kt in range(CT):
                        nc.tensor.matmul(acc, wts["pw"][:, kt, mt * 128:(mt + 1) * 128],
                                         o_norm[:, kt, icol * 512:(icol + 1) * 512],
                                         start=False, stop=(kt == CT - 1))
                    x_r = xres.tile([128, 512], F32, tag="xr")
                    nc.sync.dma_start(
                        out=x_r,
                        in_=x_d[mt * 128:(mt + 1) * 128, icol * 512:(icol + 1) * 512])
                    o_t = pj_sb.tile([128, 512], F32, tag="ot")
                    nc.vector.tensor_add(o_t, acc, x_r)
                    nc.sync.dma_start(
                        out=out_d[mt * 128:(mt + 1) * 128, icol * 512:(icol + 1) * 512],
                        in_=o_t)


# ---------------------------------------------------------------------------
# host side
# ---------------------------------------------------------------------------

_CACHE = {}


def _get_nc(reps=1):
    if reps not in _CACHE:
        _CACHE[reps] = build_kernel(reps)
    return _CACHE[reps]


def _make_in_maps(inputs):
    x = np.ascontiguousarray(np.asarray(inputs["x"], dtype=np.float32))
    qkv_w = np.asarray(inputs["qkv_w"], dtype=np.float32)
    qkv_b = np.asarray(inputs["qkv_b"], dtype=np.float32)
    proj_w = np.asarray(inputs["proj_w"], dtype=np.float32)
    proj_b = np.asarray(inputs["proj_b"], dtype=np.float32)
    norm_w = np.asarray(inputs["norm_w"], dtype=np.float32)
    norm_b = np.asarray(inputs["norm_b"], dtype=np.float32)

    shared = {
        "qw_t": np.ascontiguousarray(qkv_w[0:C].T),
        "kw_t": np.ascontiguousarray(qkv_w[C:2 * C].T),
        "vw_t": np.ascontiguousarray(qkv_w[2 * C:3 * C].T),
        "pw_t": np.ascontiguousarray(proj_w.T),
        "biases": np.ascontiguousarray(
            np.stack([qkv_b[0:C], qkv_b[C:2 * C], qkv_b[2 * C:3 * C], proj_b])),
        "nwb": np.ascontiguousarray(np.stack([norm_w, norm_b])),
        "mask8": np.ascontiguousarray(
            (np.arange(128)[:, None] // GSIZE == np.arange(8)[None, :]).astype(np.float32)),
        "mask8t": np.ascontiguousarray(
            (np.arange(128)[None, :] // GSIZE == np.arange(8)[:, None]).astype(np.float32)),
    }

    in_maps = []
    for core in range(8):
        b, ih = core // 2, core % 2
        xb = x[b].reshape(C, S)
        if ih == 1:
            xb = np.concatenate([xb[:, IHALF:], xb[:, :IHALF]], axis=1)
        m = dict(shared)
        m["x"] = np.ascontiguousarray(xb)
        in_maps.append(m)
    return in_maps


def kernel(**inputs):
    nc = _get_nc(1)
    in_maps = _make_in_maps(inputs)
    res = run_bass_kernel_spmd(nc, in_maps, core_ids=list(range(8)))
    y = np.empty((B, C, S), dtype=np.float32)
    for core in range(8):
        b, ih = core // 2, core % 2
        y[b][:, ih * IHALF:(ih + 1) * IHALF] = res.results[core]["out"]
    return y.reshape(B, C, HH, WW)


# revision 3
# speedup vs baseline: 1.4613x; 1.4613x over previous
"""Trainium2 Bass kernel for nn_AttentionBlock (GroupNorm + 1x1-conv QKV +
softmax attention + 1x1-conv proj + residual), B=4 C=512 H=W=64 HEADS=8.

Sharding: 8 cores = (batch b in 0..4) x (query-half ih in 0..2).  Each core
computes groupnorm + K/V for its whole batch (duplicated across the 2 cores
sharing a batch -- cheap) and attention + proj for its 2048 queries.  Cores
are fully independent SPMD (no collectives); the host splits and concats.
For ih=1 cores the host *rolls* the spatial columns of x by 2048 so that the
query half is always columns [0:2048): groupnorm statistics and the
attention key-sum are permutation-invariant over the spatial axis, so the
math is unchanged.

Per-(b,h) attention uses a "scores transposed" layout S^T[j,i] (keys j on
partitions, queries i on free dim): lhsT=K_h[64d,128j], rhs=Q_h[64d,512i].
Softmax is over the partition dim: exp on ScalarE (PSUM->SBUF, 1/8 scale
folded in; no max subtraction -- logits are bounded ~|1.5| here), column
sums come free from a ones-row appended to the V^T stationary operand
(output row 64 = colsum), and 1/colsum is applied to O via a ones-matmul
broadcast + DVE multiply.  Matmul operands are bf16 (f32 PSUM accumulate):
simulated end-to-end quantization error is ~2e-5 relative.
"""

from contextlib import ExitStack

import numpy as np

import concourse.bass as bass
import concourse.tile as tile
import concourse.mybir as mybir
from concourse import bacc
from concourse.bass_utils import run_bass_kernel_spmd

F32 = mybir.dt.float32
F32R = mybir.dt.float32r
BF16 = mybir.dt.bfloat16
EXP = mybir.ActivationFunctionType.Exp
IDENT = mybir.ActivationFunctionType.Identity
SQRT = mybir.ActivationFunctionType.Sqrt

B, C, HH, WW = 4, 512, 64, 64
S = HH * WW              # 4096
HEADS = 8
HD = C // HEADS          # 64
GROUPS = 32
GSIZE = C // GROUPS      # 16 channels per group
EPS = 1e-5
SCALE = 1.0 / 8.0        # 1/sqrt(head_dim)
IHALF = S // 2           # 2048 queries per core
CT = C // 128            # 4 channel tiles
ST = S // 128            # 32 spatial tiles


def build_kernel(reps: int = 1):
    nc = bacc.Bacc("TRN2", target_bir_lowering=False, debug=False)

    x_d = nc.dram_tensor("x", [C, S], F32, kind="ExternalInput").ap()
    qw_d = nc.dram_tensor("qw_t", [C, C], F32, kind="ExternalInput").ap()   # qkv_w[0:512].T
    kw_d = nc.dram_tensor("kw_t", [C, C], F32, kind="ExternalInput").ap()
    vw_d = nc.dram_tensor("vw_t", [C, C], F32, kind="ExternalInput").ap()
    pw_d = nc.dram_tensor("pw_t", [C, C], F32, kind="ExternalInput").ap()   # proj_w.T
    bias_d = nc.dram_tensor("biases", [4, C], F32, kind="ExternalInput").ap()  # qb,kb,vb,pb
    nwb_d = nc.dram_tensor("nwb", [2, C], F32, kind="ExternalInput").ap()      # norm_w, norm_b
    m8_d = nc.dram_tensor("mask8", [128, 8], F32, kind="ExternalInput").ap()
    m8t_d = nc.dram_tensor("mask8t", [8, 128], F32, kind="ExternalInput").ap()
    out_d = nc.dram_tensor("out", [C, IHALF], F32, kind="ExternalOutput").ap()

    with tile.TileContext(nc) as tc:
        with ExitStack() as ctx:
            const = ctx.enter_context(tc.tile_pool(name="const", bufs=1))

            # weight matrices, rearranged [128, kt, c], rounded to bf16
            wts = {}
            with tc.tile_pool(name="wtmp", bufs=2) as wtmp:
                for nm, ap in [("qw", qw_d), ("kw", kw_d), ("vw", vw_d), ("pw", pw_d)]:
                    t_f = wtmp.tile([128, CT, C], F32, tag="wf", name=f"{nm}_f32")
                    nc.sync.dma_start(out=t_f, in_=ap.rearrange("(kt p) c -> p kt c", p=128))
                    t_b = const.tile([128, CT, C], BF16, name=f"{nm}_bf")
                    nc.vector.tensor_copy(t_b, t_f)
                    wts[nm] = t_b

            bias_f = const.tile([1, 4, C], F32)
            nc.sync.dma_start(out=bias_f, in_=bias_d[None, :, :])
            bias_b = const.tile([1, 4, C], BF16)
            nc.vector.tensor_copy(bias_b, bias_f)

            nwb_t = const.tile([128, 2, CT], F32)
            nc.sync.dma_start(out=nwb_t, in_=nwb_d.rearrange("w (kt p) -> p w kt", p=128))

            m8_t = const.tile([128, 8], F32)
            nc.sync.dma_start(out=m8_t, in_=m8_d)
            m8t_t = const.tile([8, 128], F32)
            nc.sync.dma_start(out=m8t_t, in_=m8t_d)

            ones_row = const.tile([1, 512], BF16)
            nc.vector.memset(ones_row, 1.0)
            ones_f = const.tile([1, 64], F32)
            nc.vector.memset(ones_f, 1.0)
            ones_row_r = const.tile([1, 64], F32R)
            nc.vector.tensor_copy(ones_row_r, ones_f)
            eps_t = const.tile([8, 1], F32)
            nc.vector.memset(eps_t, EPS)

            consts = (wts, bias_b, nwb_t, m8_t, m8t_t, ones_row, ones_row_r, eps_t)
            if reps == 1:
                _one_pass(nc, tc, x_d, out_d, consts)
            else:
                with tc.For_i(0, reps, 1):
                    _one_pass(nc, tc, x_d, out_d, consts)
    nc.compile()
    return nc


def _one_pass(nc, tc, x_d, out_d, consts):
    (wts, bias_b, nwb_t, m8_t, m8t_t, ones_row, ones_row_r, eps_t) = consts
    with ExitStack() as ps:
        # =============== P1: groupnorm statistics ===============
        svec = ps.enter_context(tc.tile_pool(name="svec", bufs=1))
        aff_s = svec.tile([128, CT], F32)   # per-channel scale
        aff_t = svec.tile([128, CT], F32)   # per-channel shift

        with tc.tile_pool(name="xin", bufs=2) as xin, \
             tc.tile_pool(name="stat", bufs=2) as stat, \
             tc.tile_pool(name="gn_ps", bufs=1, space="PSUM") as gn_ps:
            gstats = gn_ps.tile([8, CT, 2], F32)
            for t in range(CT):
                x_t = xin.tile([128, S], F32, tag="x")
                nc.sync.dma_start(out=x_t, in_=x_d[t * 128:(t + 1) * 128, :])
                bnst = stat.tile([128, 8, 6], F32, tag="bnst")
                for sg in range(8):
                    nc.vector.bn_stats(out=bnst[:, sg, :], in_=x_t[:, sg * 512:(sg + 1) * 512])
                mv = stat.tile([128, 2], F32, tag="mv")
                nc.vector.bn_aggr(out=mv, in_=bnst)
                st_t = stat.tile([128, 2], F32, tag="st")   # (mean, 2nd moment)
                nc.vector.tensor_copy(st_t[:, 0:1], mv[:, 0:1])
                sq = stat.tile([128, 1], F32, tag="sq")
                nc.vector.tensor_mul(sq, mv[:, 0:1], mv[:, 0:1])
                nc.vector.tensor_add(st_t[:, 1:2], mv[:, 1:2], sq)
                nc.tensor.matmul(gstats[:, t, :], m8_t, st_t, start=True, stop=True)

            gs = stat.tile([8, CT, 2], F32, tag="gs")
            nc.vector.tensor_copy(gs, gstats)
            grp = stat.tile([8, CT, 2], F32, tag="grp")   # (mu_g, rstd_g)
            for t in range(CT):
                nc.scalar.mul(grp[:, t, 0:1], gs[:, t, 0:1], 1.0 / GSIZE)
                e_t = stat.tile([8, 1], F32, tag="e")
                nc.scalar.mul(e_t, gs[:, t, 1:2], 1.0 / GSIZE)
                musq = stat.tile([8, 1], F32, tag="musq")
                nc.vector.tensor_mul(musq, grp[:, t, 0:1], grp[:, t, 0:1])
                var = stat.tile([8, 1], F32, tag="var")
                nc.vector.tensor_sub(var, e_t, musq)
                std = stat.tile([8, 1], F32, tag="std")
                nc.scalar.activation(std, var, SQRT, bias=eps_t)
                nc.vector.reciprocal(grp[:, t, 1:2], std)

            bc_ps = gn_ps.tile([128, CT, 2], F32)
            for t in range(CT):
                nc.tensor.matmul(bc_ps[:, t, :], m8t_t, grp[:, t, :], start=True, stop=True)
            bc = stat.tile([128, CT, 2], F32, tag="bc")
            nc.vector.tensor_copy(bc, bc_ps)
            # aff_s = norm_w * rstd ; aff_t = norm_b - mu * aff_s
            nc.vector.tensor_mul(aff_s, nwb_t[:, 0, :], bc[:, :, 1])
            tmp_mu = stat.tile([128, CT], F32, tag="tmpmu")
            nc.vector.tensor_mul(tmp_mu, bc[:, :, 0], aff_s)
            nc.vector.tensor_sub(aff_t, nwb_t[:, 1, :], tmp_mu)

        # =============== P2: normalize -> h (bf16), qkv projections ===============
        big = ps.enter_context(tc.tile_pool(name="big", bufs=1))
        h_all = big.tile([128, CT, S], BF16)           # 32 KB/part
        k_all = big.tile([128, CT, S], BF16)           # 32 KB/part
        q_all = big.tile([128, CT, IHALF], BF16)       # 16 KB/part
        vt_ext = big.tile([128, ST, HEADS, 66], BF16)  # ~33 KB/part
        o_norm = big.tile([128, CT, IHALF], BF16)      # 16 KB/part

        with tc.tile_pool(name="xin2", bufs=2) as xin2:
            for t in range(CT):
                x_t = xin2.tile([128, S], F32, tag="x2")
                nc.sync.dma_start(out=x_t, in_=x_d[t * 128:(t + 1) * 128, :])
                nc.scalar.activation(h_all[:, t, :], x_t, IDENT,
                                     bias=aff_t[:, t:t + 1], scale=aff_s[:, t:t + 1])

        nc.vector.memset(vt_ext[:, :, :, 64:66], 0.0)
        nc.vector.memset(vt_ext[:, :, :, 64:65], 1.0)

        with tc.tile_pool(name="qkv_ps", bufs=2, space="PSUM") as qkv_ps, \
             tc.tile_pool(name="qkv_sb", bufs=3) as qkv_sb:
            del qkv_sb
            # K: [512c, S] and Q: [512c, IHALF] (query half = cols [0:2048))
            for dst, w_b, bidx, ncols in ((k_all, wts["kw"], 1, S),
                                          (q_all, wts["qw"], 0, IHALF)):
                for mt in range(CT):
                    for icol in range(ncols // 512):
                        acc = qkv_ps.tile([128, 512], F32, tag="qkps")
                        nc.tensor.matmul(acc, bias_b[:, bidx, mt * 128:(mt + 1) * 128],
                                         ones_row, start=True, stop=False)
                        for kt in range(CT):
                            nc.tensor.matmul(acc, w_b[:, kt, mt * 128:(mt + 1) * 128],
                                             h_all[:, kt, icol * 512:(icol + 1) * 512],
                                             start=False, stop=(kt == CT - 1))
                        nc.vector.tensor_copy(dst[:, mt, icol * 512:(icol + 1) * 512], acc)
            # V^T: [S, 512c] scattered into vt_ext (64 cols per head + ones col)
            for st in range(ST):
                acc = qkv_ps.tile([128, 512], F32, tag="qkps")
                nc.tensor.matmul(acc, ones_row[:, 0:128], bias_b[:, 2, :],
                                 start=True, stop=False)
                for kt in range(CT):
                    nc.tensor.matmul(acc, h_all[:, kt, st * 128:(st + 1) * 128],
                                     wts["vw"][:, kt, :], start=False, stop=(kt == CT - 1))
                nc.vector.tensor_copy(vt_ext[:, st, :, 0:64],
                                      acc.rearrange("p (h d) -> p h d", d=HD))

        # =============== P3: attention ===============
        with tc.tile_pool(name="sc_ps", bufs=2, space="PSUM") as sc_psp, \
             tc.tile_pool(name="o_ps", bufs=1, space="PSUM") as o_psp, \
             tc.tile_pool(name="rb_ps", bufs=1, space="PSUM") as rb_psp, \
             tc.tile_pool(name="att_sb", bufs=3) as att_sb, \
             tc.tile_pool(name="exp_sb", bufs=3) as exp_sb:
            for h in range(HEADS):
                kt_h = h // 2
                p0 = 64 * (h % 2)
                for iw in range(2):
                    o_ps = o_psp.tile([65, 1024], F32, tag="ops")

                    # software-pipelined by one jt: PE program order is
                    # sc(jt+1) then O(jt), so the scores matmuls for the next
                    # tile run while ScalarE computes exp(jt) -- otherwise the
                    # in-order PE would stall behind the exp dependency.
                    def emit_o(jt, ex):
                        for ic in range(2):
                            nc.tensor.matmul(
                                o_ps[:, ic * 512:(ic + 1) * 512],
                                vt_ext[:, jt, h, 0:65],
                                ex[:, ic * 512:(ic + 1) * 512],
                                start=(jt == 0), stop=(jt == ST - 1),
                                skip_group_check=True)

                    prev = None  # (jt, ex)
                    for jt in range(ST):
                        sc = sc_psp.tile([128, 1024], F32, tag="sc")
                        for ic in range(2):
                            nc.tensor.matmul(
                                sc[:, ic * 512:(ic + 1) * 512],
                                k_all[p0:p0 + 64, kt_h, jt * 128:(jt + 1) * 128],
                                q_all[p0:p0 + 64, kt_h,
                                      iw * 1024 + ic * 512: iw * 1024 + (ic + 1) * 512],
                                start=True, stop=True)
                        if prev is not None:
                            emit_o(*prev)
                        ex = exp_sb.tile([128, 1024], BF16, tag="ex")
                        nc.scalar.activation(ex, sc, EXP, scale=SCALE)
                        prev = (jt, ex)
                    emit_o(*prev)
                    # normalize by colsum (row 64)
                    o_sb = att_sb.tile([65, 1024], F32, tag="osb")
                    nc.vector.tensor_copy(o_sb, o_ps)
                    r_f = att_sb.tile([1, 1024], F32, tag="rf")
                    nc.vector.reciprocal(r_f, o_sb[64:65, :])
                    r_r = att_sb.tile([1, 1024], F32R, tag="rr")
                    nc.vector.tensor_copy(r_r, r_f)
                    rb = rb_psp.tile([64, 1024], F32, tag="rb")
                    for ic in range(2):
                        nc.tensor.matmul(rb[:, ic * 512:(ic + 1) * 512], ones_row_r,
                                         r_r[:, ic * 512:(ic + 1) * 512],
                                         start=True, stop=True)
                    nc.vector.tensor_mul(
                        o_norm[p0:p0 + 64, kt_h, iw * 1024:(iw + 1) * 1024],
                        o_sb[0:64, :], rb)

        # =============== P4: proj + residual ===============
        with tc.tile_pool(name="pj_ps", bufs=2, space="PSUM") as pj_ps, \
             tc.tile_pool(name="pj_sb", bufs=3) as pj_sb, \
             tc.tile_pool(name="xres", bufs=3) as xres:
            for mt in range(CT):
                for icol in range(IHALF // 512):
                    acc = pj_ps.tile([128, 512], F32, tag="pj")
                    nc.tensor.matmul(acc, bias_b[:, 3, mt * 128:(mt + 1) * 128],
                                     ones_row, start=True, stop=False)
                    for kt in range(CT):
                        nc.tensor.matmul(acc, wts["pw"][:, kt, mt * 128:(mt + 1) * 128],
                                         o_norm[:, kt, icol * 512:(icol + 1) * 512],
                                         start=False, stop=(kt == CT - 1))
                    x_r = xres.tile([128, 512], F32, tag="xr")
                    nc.sync.dma_start(
                        out=x_r,
                        in_=x_d[mt * 128:(mt + 1) * 128, icol * 512:(icol + 1) * 512])
                    o_t = pj_sb.tile([128, 512], F32, tag="ot")
                    nc.vector.tensor_add(o_t, acc, x_r)
                    nc.sync.dma_start(
                        out=out_d[mt * 128:(mt + 1) * 128, icol * 512:(icol + 1) * 512],
                        in_=o_t)


# ---------------------------------------------------------------------------
# host side
# ---------------------------------------------------------------------------

_CACHE = {}


def _get_nc(reps=1):
    if reps not in _CACHE:
        _CACHE[reps] = build_kernel(reps)
    return _CACHE[reps]


def _make_in_maps(inputs):
    x = np.ascontiguousarray(np.asarray(inputs["x"], dtype=np.float32))
    qkv_w = np.asarray(inputs["qkv_w"], dtype=np.float32)
    qkv_b = np.asarray(inputs["qkv_b"], dtype=np.float32)
    proj_w = np.asarray(inputs["proj_w"], dtype=np.float32)
    proj_b = np.asarray(inputs["proj_b"], dtype=np.float32)
    norm_w = np.asarray(inputs["norm_w"], dtype=np.float32)
    norm_b = np.asarray(inputs["norm_b"], dtype=np.float32)

    shared = {
        "qw_t": np.ascontiguousarray(qkv_w[0:C].T),
        "kw_t": np.ascontiguousarray(qkv_w[C:2 * C].T),
        "vw_t": np.ascontiguousarray(qkv_w[2 * C:3 * C].T),
        "pw_t": np.ascontiguousarray(proj_w.T),
        "biases": np.ascontiguousarray(
            np.stack([qkv_b[0:C], qkv_b[C:2 * C], qkv_b[2 * C:3 * C], proj_b])),
        "nwb": np.ascontiguousarray(np.stack([norm_w, norm_b])),
        "mask8": np.ascontiguousarray(
            (np.arange(128)[:, None] // GSIZE == np.arange(8)[None, :]).astype(np.float32)),
        "mask8t": np.ascontiguousarray(
            (np.arange(128)[None, :] // GSIZE == np.arange(8)[:, None]).astype(np.float32)),
    }

    in_maps = []
    for core in range(8):
        b, ih = core // 2, core % 2
        xb = x[b].reshape(C, S)
        if ih == 1:
            xb = np.concatenate([xb[:, IHALF:], xb[:, :IHALF]], axis=1)
        m = dict(shared)
        m["x"] = np.ascontiguousarray(xb)
        in_maps.append(m)
    return in_maps


def kernel(**inputs):
    nc = _get_nc(1)
    in_maps = _make_in_maps(inputs)
    res = run_bass_kernel_spmd(nc, in_maps, core_ids=list(range(8)))
    y = np.empty((B, C, S), dtype=np.float32)
    for core in range(8):
        b, ih = core // 2, core % 2
        y[b][:, ih * IHALF:(ih + 1) * IHALF] = res.results[core]["out"]
    return y.reshape(B, C, HH, WW)


# revision 10
# speedup vs baseline: 1.6816x; 1.1508x over previous
"""Trainium2 Bass kernel for nn_AttentionBlock (GroupNorm + 1x1-conv QKV +
softmax attention + 1x1-conv proj + residual), B=4 C=512 H=W=64 HEADS=8.

Sharding: 8 cores = (batch b in 0..4) x (query-half ih in 0..2).  Each core
computes groupnorm + K/V for its whole batch (duplicated across the 2 cores
sharing a batch -- cheap) and attention + proj for its 2048 queries.  Cores
are fully independent SPMD (no collectives); the host splits and concats.
For ih=1 cores the host *rolls* the spatial columns of x by 2048 so that the
query half is always columns [0:2048): groupnorm statistics and the
attention key-sum are permutation-invariant over the spatial axis, so the
math is unchanged.

Per-(b,h) attention uses a "scores transposed" layout S^T[j,i] (keys j on
partitions, queries i on free dim): lhsT=K_h[64d,128j], rhs=Q_h[64d,512i].
Softmax is over the partition dim: exp on ScalarE (PSUM->SBUF, 1/8 scale
folded in; no max subtraction -- logits are bounded ~|1.5| here), column
sums come free from a ones-row appended to the V^T stationary operand
(output row 64 = colsum), and 1/colsum is applied to O via a ones-matmul
broadcast + DVE multiply.  Matmul operands are bf16 (f32 PSUM accumulate):
simulated end-to-end quantization error is ~2e-5 relative.
"""

from contextlib import ExitStack

import numpy as np

import concourse.bass as bass
import concourse.tile as tile
import concourse.mybir as mybir
from concourse import bacc
from concourse.bass_utils import run_bass_kernel_spmd

F32 = mybir.dt.float32
F32R = mybir.dt.float32r
BF16 = mybir.dt.bfloat16
EXP = mybir.ActivationFunctionType.Exp
IDENT = mybir.ActivationFunctionType.Identity
SQRT = mybir.ActivationFunctionType.Sqrt

B, C, HH, WW = 4, 512, 64, 64
S = HH * WW              # 4096
HEADS = 8
HD = C // HEADS          # 64
GROUPS = 32
GSIZE = C // GROUPS      # 16 channels per group
EPS = 1e-5
SCALE = 1.0 / 8.0        # 1/sqrt(head_dim)
IHALF = S // 2           # 2048 queries per core
CT = C // 128            # 4 channel tiles
ST = S // 128            # 32 spatial tiles
ATT_MODE = "pair"


def build_kernel(reps: int = 1):
    nc = bacc.Bacc("TRN2", target_bir_lowering=False, debug=False)

    x_d = nc.dram_tensor("x", [C, S], F32, kind="ExternalInput").ap()
    qw_d = nc.dram_tensor("qw_t", [C, C], F32, kind="ExternalInput").ap()   # qkv_w[0:512].T
    kw_d = nc.dram_tensor("kw_t", [C, C], F32, kind="ExternalInput").ap()
    vw_d = nc.dram_tensor("vw_t", [C, C], F32, kind="ExternalInput").ap()
    pw_d = nc.dram_tensor("pw_t", [C, C], F32, kind="ExternalInput").ap()   # proj_w.T
    bias_d = nc.dram_tensor("biases", [4, C], F32, kind="ExternalInput").ap()  # qb,kb,vb,pb
    nwb_d = nc.dram_tensor("nwb", [2, C], F32, kind="ExternalInput").ap()      # norm_w, norm_b
    m8_d = nc.dram_tensor("mask8", [128, 8], F32, kind="ExternalInput").ap()
    m8t_d = nc.dram_tensor("mask8t", [8, 128], F32, kind="ExternalInput").ap()
    out_d = nc.dram_tensor("out", [C, IHALF], F32, kind="ExternalOutput").ap()

    with tile.TileContext(nc) as tc:
        with ExitStack() as ctx:
            const = ctx.enter_context(tc.tile_pool(name="const", bufs=1))

            # weight matrices, rearranged [128, kt, c], rounded to bf16
            wts = {}
            with tc.tile_pool(name="wtmp", bufs=2) as wtmp:
                for nm, ap in [("qw", qw_d), ("kw", kw_d), ("vw", vw_d), ("pw", pw_d)]:
                    t_f = wtmp.tile([128, CT, C], F32, tag="wf", name=f"{nm}_f32")
                    nc.sync.dma_start(out=t_f, in_=ap.rearrange("(kt p) c -> p kt c", p=128))
                    t_b = const.tile([128, CT, C], BF16, name=f"{nm}_bf")
                    nc.vector.tensor_copy(t_b, t_f)
                    wts[nm] = t_b

            bias_cols = const.tile([128, 4, CT], F32)
            nc.sync.dma_start(out=bias_cols,
                              in_=bias_d.rearrange("w (kt p) -> p w kt", p=128))
            vb_bcast = const.tile([128, C], F32)
            nc.sync.dma_start(out=vb_bcast, in_=bias_d[2:3, :].to_broadcast([128, C]))

            nwb_t = const.tile([128, 2, CT], F32)
            nc.sync.dma_start(out=nwb_t, in_=nwb_d.rearrange("w (kt p) -> p w kt", p=128))

            m8_t = const.tile([128, 8], F32)
            nc.sync.dma_start(out=m8_t, in_=m8_d)
            m8t_t = const.tile([8, 128], F32)
            nc.sync.dma_start(out=m8t_t, in_=m8t_d)

            ones_f = const.tile([1, 64], F32)
            nc.vector.memset(ones_f, 1.0)
            ones_row_r = const.tile([1, 64], F32R)
            nc.vector.tensor_copy(ones_row_r, ones_f)
            eps_t = const.tile([8, 1], F32)
            nc.vector.memset(eps_t, EPS)

            consts = (wts, bias_cols, vb_bcast, nwb_t, m8_t, m8t_t,
                      ones_row_r, eps_t)
            if reps == 1:
                _one_pass(nc, tc, x_d, out_d, consts)
            else:
                with tc.For_i(0, reps, 1):
                    _one_pass(nc, tc, x_d, out_d, consts)
    nc.compile()
    return nc


def _one_pass(nc, tc, x_d, out_d, consts):
    (wts, bias_cols, vb_bcast, nwb_t, m8_t, m8t_t,
     ones_row_r, eps_t) = consts
    with ExitStack() as ps:
        # =============== P1: groupnorm statistics ===============
        svec = ps.enter_context(tc.tile_pool(name="svec", bufs=1))
        aff_s = svec.tile([128, CT], F32)   # per-channel scale
        aff_t = svec.tile([128, CT], F32)   # per-channel shift

        with tc.tile_pool(name="xin", bufs=2) as xin, \
             tc.tile_pool(name="stat", bufs=2) as stat, \
             tc.tile_pool(name="gn_ps", bufs=1, space="PSUM") as gn_ps:
            gstats = gn_ps.tile([8, CT, 2], F32)
            for t in range(CT):
                x_t = xin.tile([128, S], F32, tag="x")
                nc.sync.dma_start(out=x_t, in_=x_d[t * 128:(t + 1) * 128, :])
                bnst = stat.tile([128, 8, 6], F32, tag="bnst")
                for sg in range(8):
                    nc.vector.bn_stats(out=bnst[:, sg, :], in_=x_t[:, sg * 512:(sg + 1) * 512])
                mv = stat.tile([128, 2], F32, tag="mv")
                nc.vector.bn_aggr(out=mv, in_=bnst)
                st_t = stat.tile([128, 2], F32, tag="st")   # (mean, 2nd moment)
                nc.vector.tensor_copy(st_t[:, 0:1], mv[:, 0:1])
                sq = stat.tile([128, 1], F32, tag="sq")
                nc.vector.tensor_mul(sq, mv[:, 0:1], mv[:, 0:1])
                nc.vector.tensor_add(st_t[:, 1:2], mv[:, 1:2], sq)
                nc.tensor.matmul(gstats[:, t, :], m8_t, st_t, start=True, stop=True)

            gs = stat.tile([8, CT, 2], F32, tag="gs")
            nc.vector.tensor_copy(gs, gstats)
            grp = stat.tile([8, CT, 2], F32, tag="grp")   # (mu_g, rstd_g)
            for t in range(CT):
                nc.scalar.mul(grp[:, t, 0:1], gs[:, t, 0:1], 1.0 / GSIZE)
                e_t = stat.tile([8, 1], F32, tag="e")
                nc.scalar.mul(e_t, gs[:, t, 1:2], 1.0 / GSIZE)
                musq = stat.tile([8, 1], F32, tag="musq")
                nc.vector.tensor_mul(musq, grp[:, t, 0:1], grp[:, t, 0:1])
                var = stat.tile([8, 1], F32, tag="var")
                nc.vector.tensor_sub(var, e_t, musq)
                std = stat.tile([8, 1], F32, tag="std")
                nc.scalar.activation(std, var, SQRT, bias=eps_t)
                nc.vector.reciprocal(grp[:, t, 1:2], std)

            bc_ps = gn_ps.tile([128, CT, 2], F32)
            for t in range(CT):
                nc.tensor.matmul(bc_ps[:, t, :], m8t_t, grp[:, t, :], start=True, stop=True)
            bc = stat.tile([128, CT, 2], F32, tag="bc")
            nc.vector.tensor_copy(bc, bc_ps)
            # aff_s = norm_w * rstd ; aff_t = norm_b - mu * aff_s
            nc.vector.tensor_mul(aff_s, nwb_t[:, 0, :], bc[:, :, 1])
            tmp_mu = stat.tile([128, CT], F32, tag="tmpmu")
            nc.vector.tensor_mul(tmp_mu, bc[:, :, 0], aff_s)
            nc.vector.tensor_sub(aff_t, nwb_t[:, 1, :], tmp_mu)

        # =============== P2: normalize -> h (bf16), qkv projections ===============
        big = ps.enter_context(tc.tile_pool(name="big", bufs=1))
        h_all = big.tile([128, CT, S], BF16)           # 32 KB/part
        k_all = big.tile([128, CT, S], BF16)           # 32 KB/part
        q_all = big.tile([128, CT, IHALF], BF16)       # 16 KB/part
        vt_ext = big.tile([128, ST, HEADS, 66], BF16)  # ~33 KB/part
        o_norm = big.tile([128, CT, IHALF], BF16)      # 16 KB/part

        with tc.tile_pool(name="xin2", bufs=2) as xin2:
            for t in range(CT):
                x_t = xin2.tile([128, S], F32, tag="x2")
                nc.sync.dma_start(out=x_t, in_=x_d[t * 128:(t + 1) * 128, :])
                nc.vector.tensor_scalar(
                    out=h_all[:, t, :], in0=x_t,
                    scalar1=aff_s[:, t:t + 1], scalar2=aff_t[:, t:t + 1],
                    op0=mybir.AluOpType.mult, op1=mybir.AluOpType.add)

        nc.vector.memset(vt_ext[:, :, :, 64:66], 0.0)
        nc.vector.memset(vt_ext[:, :, :, 64:65], 1.0)

        with tc.tile_pool(name="qkv_ps", bufs=2, space="PSUM") as qkv_ps, \
             tc.tile_pool(name="qkv_sb", bufs=3) as qkv_sb:
            del qkv_sb
            # K: [512c, S] and Q: [512c, IHALF] (query half = cols [0:2048))
            for dst, w_b, bidx, ncols in ((k_all, wts["kw"], 1, S),
                                          (q_all, wts["qw"], 0, IHALF)):
                for mt in range(CT):
                    for icol in range(ncols // 512):
                        acc = qkv_ps.tile([128, 512], F32, tag="qkps")
                        for kt in range(CT):
                            nc.tensor.matmul(acc, w_b[:, kt, mt * 128:(mt + 1) * 128],
                                             h_all[:, kt, icol * 512:(icol + 1) * 512],
                                             start=(kt == 0), stop=(kt == CT - 1))
                        nc.vector.tensor_scalar_add(
                            out=dst[:, mt, icol * 512:(icol + 1) * 512], in0=acc,
                            scalar1=bias_cols[:, bidx, mt:mt + 1])
            # V^T: [S, 512c] scattered into vt_ext (64 cols per head + ones col)
            for st in range(ST):
                acc = qkv_ps.tile([128, 512], F32, tag="qkps")
                for kt in range(CT):
                    nc.tensor.matmul(acc, h_all[:, kt, st * 128:(st + 1) * 128],
                                     wts["vw"][:, kt, :], start=(kt == 0), stop=(kt == CT - 1))
                nc.vector.tensor_add(vt_ext[:, st, :, 0:64],
                                     acc.rearrange("p (h d) -> p h d", d=HD),
                                     vb_bcast.rearrange("p (h d) -> p h d", d=HD))

        # =============== P3: attention ===============
        # Heads are processed in pairs (2t, 2t+1): the even head lives on
        # partitions 0:64 of k_all/q_all, the odd head on 64:128, so their
        # K=64 score matmuls occupy disjoint PE row-groups and execute
        # CONCURRENTLY (tile_position row packing, ~2x on the score GEMMs).
        # O matmuls are software-pipelined one jt behind the scores so the
        # in-order PE never stalls on the exp dependency.
        with tc.tile_pool(name="att_ps", bufs=1, space="PSUM") as att_ps, \
             tc.tile_pool(name="att_sb", bufs=3) as att_sb, \
             tc.tile_pool(name="exp_sb", bufs=2) as exp_sb:
            for hp in range(CT):
                for iw in range(2):
                    o_ps = [att_ps.tile([65, 1024], F32, tag=f"o{p}", name=f"o_ps{p}")
                            for p in range(2)]

                    def emit_sc(jt, p):
                        sc = att_ps.tile([128, 1024], F32, tag=f"sc{p}", name=f"sc{p}")
                        for ic in range(2):
                            nc.tensor.matmul(
                                sc[:, ic * 512:(ic + 1) * 512],
                                k_all[64 * p:64 * p + 64, hp, jt * 128:(jt + 1) * 128],
                                q_all[64 * p:64 * p + 64, hp,
                                      iw * 1024 + ic * 512: iw * 1024 + (ic + 1) * 512],
                                start=True, stop=True)
                        return sc

                    def emit_exp(sc, p):
                        ex = exp_sb.tile([128, 1024], BF16, tag=f"ex{p}", name=f"ex{p}")
                        nc.scalar.activation(ex, sc, EXP, scale=SCALE)
                        return ex

                    def emit_o(jt, ex, p):
                        for ic in range(2):
                            nc.tensor.matmul(
                                o_ps[p][:, ic * 512:(ic + 1) * 512],
                                vt_ext[:, jt, 2 * hp + p, 0:65],
                                ex[:, ic * 512:(ic + 1) * 512],
                                start=(jt == 0), stop=(jt == ST - 1),
                                skip_group_check=True)

                    if ATT_MODE == "pair":
                        prev = None
                        for jt in range(ST):
                            scs = [emit_sc(jt, p) for p in range(2)]
                            if prev is not None:
                                for p in range(2):
                                    emit_o(prev[0], prev[1][p], p)
                            exs = [emit_exp(scs[p], p) for p in range(2)]
                            prev = (jt, exs)
                        for p in range(2):
                            emit_o(prev[0], prev[1][p], p)
                    elif ATT_MODE == "pair_nodelay":
                        for jt in range(ST):
                            scs = [emit_sc(jt, p) for p in range(2)]
                            exs = [emit_exp(scs[p], p) for p in range(2)]
                            for p in range(2):
                                emit_o(jt, exs[p], p)
                    else:  # "seq"
                        for p in range(2):
                            prev = None
                            for jt in range(ST):
                                sc = emit_sc(jt, p)
                                if prev is not None:
                                    emit_o(prev[0], prev[1], p)
                                ex = emit_exp(sc, p)
                                prev = (jt, ex)
                            emit_o(prev[0], prev[1], p)

                    # normalize by colsum (row 64 of o_ps)
                    for p in range(2):
                        o_sb = att_sb.tile([65, 1024], F32, tag=f"osb{p}", name=f"osb{p}", bufs=2)
                        nc.vector.tensor_copy(o_sb, o_ps[p])
                        cs = att_sb.tile([1, 1024], F32, tag=f"cs{p}", name=f"cs{p}", bufs=1)
                        nc.vector.tensor_copy(cs, o_sb[64:65, :])
                        r_f = att_sb.tile([1, 1024], F32, tag=f"rf{p}", name=f"rf{p}", bufs=1)
                        nc.vector.reciprocal_approx_fast(out=r_f, in_=cs)
                        rb = att_sb.tile([64, 1024], F32, tag=f"rb{p}", name=f"rb{p}", bufs=2)
                        nc.gpsimd.partition_broadcast(rb, r_f)
                        nc.vector.tensor_mul(
                            o_norm[64 * p:64 * p + 64, hp, iw * 1024:(iw + 1) * 1024],
                            o_sb[0:64, :], rb)

        # =============== P4: proj + residual ===============
        with tc.tile_pool(name="pj_ps", bufs=2, space="PSUM") as pj_ps, \
             tc.tile_pool(name="pj_sb", bufs=3) as pj_sb, \
             tc.tile_pool(name="xres", bufs=3) as xres:
            for mt in range(CT):
                for icol in range(IHALF // 512):
                    acc = pj_ps.tile([128, 512], F32, tag="pj")
                    for kt in range(CT):
                        nc.tensor.matmul(acc, wts["pw"][:, kt, mt * 128:(mt + 1) * 128],
                                         o_norm[:, kt, icol * 512:(icol + 1) * 512],
                                         start=(kt == 0), stop=(kt == CT - 1))
                    x_r = xres.tile([128, 512], F32, tag="xr")
                    nc.sync.dma_start(
                        out=x_r,
                        in_=x_d[mt * 128:(mt + 1) * 128, icol * 512:(icol + 1) * 512])
                    o_t = pj_sb.tile([128, 512], F32, tag="ot")
                    nc.vector.scalar_tensor_tensor(
                        out=o_t, in0=acc, scalar=bias_cols[:, 3, mt:mt + 1],
                        in1=x_r, op0=mybir.AluOpType.add, op1=mybir.AluOpType.add)
                    nc.sync.dma_start(
                        out=out_d[mt * 128:(mt + 1) * 128, icol * 512:(icol + 1) * 512],
                        in_=o_t)


# ---------------------------------------------------------------------------
# host side
# ---------------------------------------------------------------------------

_CACHE = {}


def _get_nc(reps=1):
    if reps not in _CACHE:
        _CACHE[reps] = build_kernel(reps)
    return _CACHE[reps]


def _make_in_maps(inputs):
    x = np.ascontiguousarray(np.asarray(inputs["x"], dtype=np.float32))
    qkv_w = np.asarray(inputs["qkv_w"], dtype=np.float32)
    qkv_b = np.asarray(inputs["qkv_b"], dtype=np.float32)
    proj_w = np.asarray(inputs["proj_w"], dtype=np.float32)
    proj_b = np.asarray(inputs["proj_b"], dtype=np.float32)
    norm_w = np.asarray(inputs["norm_w"], dtype=np.float32)
    norm_b = np.asarray(inputs["norm_b"], dtype=np.float32)

    shared = {
        "qw_t": np.ascontiguousarray(qkv_w[0:C].T),
        "kw_t": np.ascontiguousarray(qkv_w[C:2 * C].T),
        "vw_t": np.ascontiguousarray(qkv_w[2 * C:3 * C].T),
        "pw_t": np.ascontiguousarray(proj_w.T),
        "biases": np.ascontiguousarray(
            np.stack([qkv_b[0:C], qkv_b[C:2 * C], qkv_b[2 * C:3 * C], proj_b])),
        "nwb": np.ascontiguousarray(np.stack([norm_w, norm_b])),
        "mask8": np.ascontiguousarray(
            (np.arange(128)[:, None] // GSIZE == np.arange(8)[None, :]).astype(np.float32)),
        "mask8t": np.ascontiguousarray(
            (np.arange(128)[None, :] // GSIZE == np.arange(8)[:, None]).astype(np.float32)),
    }

    in_maps = []
    for core in range(8):
        b, ih = core // 2, core % 2
        xb = x[b].reshape(C, S)
        if ih == 1:
            xb = np.concatenate([xb[:, IHALF:], xb[:, :IHALF]], axis=1)
        m = dict(shared)
        m["x"] = np.ascontiguousarray(xb)
        in_maps.append(m)
    return in_maps


def kernel(**inputs):
    nc = _get_nc(1)
    in_maps = _make_in_maps(inputs)
    res = run_bass_kernel_spmd(nc, in_maps, core_ids=list(range(8)))
    y = np.empty((B, C, S), dtype=np.float32)
    for core in range(8):
        b, ih = core // 2, core % 2
        y[b][:, ih * IHALF:(ih + 1) * IHALF] = res.results[core]["out"]
    return y.reshape(B, C, HH, WW)
